# revision 11
# baseline (speedup 1.0000x reference)
"""Trainium2 Bass kernel for nn_DoubleConv (2-layer mean-aggregate SAGEConv on a
fixed periodic-grid graph).

Contract: kernel(**inputs) takes FULL unsharded inputs (as produced by
reference.setup_inputs()) and returns the FULL output [4, 6, 96, 96, 256] f32.

Strategy
--------
The reference graph is a fixed 4-connectivity periodic 96x96 grid per tile
(6 tiles, neighbors never cross tiles).  The neighbor-mean is therefore a
stencil: mean(h[nbrs]) = 0.25 * (up + down + left + right) with periodic wrap.
We verify at runtime that `neighbors` matches that grid; if it ever doesn't,
a numpy fallback computes the exact reference formula on host.

Sharding: 8 cores = 4 batches x 2 halves (3 grid-tiles each).  Tiles are
independent for the stencil, so there is no halo exchange and no redundant
compute.  Per core: 27648 nodes.

Device layout is channel-major ([C, nodes] on SBUF partitions x free dim):
  - the stencil becomes shifted adds along the free dimension,
  - matmuls chain naturally (PSUM output [C_out, nodes] is the next layer's
    moving operand),
  - host does the cheap input transpose / output untranspose in numpy.

Per layer both matmuls are fused into one K-concatenated matmul:
  h @ W_self + mean(h[nbrs]) @ W_neigh = [h ; stencil(h)] @ [W_self ; W_neigh/4]
(0.25 folded into W_neigh on host).  Matmuls run in bf16 with f32 PSUM
accumulation; biases + ReLU are applied on the scalar engine during PSUM
evacuation.
"""

import numpy as np
import ml_dtypes

# ---- problem constants (hardcoded per task contract) ----
BATCH = 4
N_TILES = 6
NX = 96
IN_C = 128
HID_C = 256
NODES_PER_TILE = NX * NX          # 9216
TILES_PER_CORE = 3
NODES_PER_CORE = TILES_PER_CORE * NODES_PER_TILE  # 27648
N_CORES = 8
CHUNK = 512                        # matmul moving-operand free dim / PSUM bank
N_CHUNKS = NODES_PER_TILE // CHUNK  # 18
GROUP = 3                          # chunks per PSUM group (3 chunks x 2 mblk = 6 banks)

_BF16 = ml_dtypes.bfloat16

_cached_nc = None


def _build_grid_neighbors():
    i, j = np.meshgrid(np.arange(NX), np.arange(NX), indexing="ij")
    idx = lambda ii, jj: (ii % NX) * NX + (jj % NX)
    per_tile = np.stack(
        [idx(i - 1, j), idx(i + 1, j), idx(i, j - 1), idx(i, j + 1)], axis=-1
    ).reshape(NX * NX, 4)
    offsets = (np.arange(N_TILES) * NX * NX)[:, None, None]
    return (per_tile[None] + offsets).reshape(-1, 4).astype(np.int32)


def _numpy_fallback(x, neighbors, W_self1, W_neigh1, b1, W_self2, W_neigh2, b2):
    B, T, X, Y, C = x.shape
    h = x.reshape(B, T * X * Y, C).astype(np.float32)
    nb = neighbors.astype(np.int64)

    def sage(h, Ws, Wn, b):
        hn = h[:, nb].mean(axis=2)
        return h @ Ws + hn @ Wn + b

    h = np.maximum(sage(h, W_self1, W_neigh1, b1), 0.0)
    h = np.maximum(sage(h, W_self2, W_neigh2, b2), 0.0)
    return h.reshape(B, T, X, Y, -1).astype(np.float32)


def _stencil(nc, mybir, out_ap, in_ap):
    """out = up + down + left + right of `in_` on a periodic NX x NX grid,
    [128, NODES_PER_TILE] channel-major, node n = i*NX + j."""
    add = mybir.AluOpType.add
    o = out_ap.rearrange("p (i j) -> p i j", j=NX)
    x = in_ap.rearrange("p (i j) -> p i j", j=NX)
    # up neighbor (i-1, wrap): out[i] = x[i-1]
    nc.vector.tensor_copy(o[:, 1:, :], x[:, : NX - 1, :])
    nc.vector.tensor_copy(o[:, 0, :], x[:, NX - 1, :])
    # down neighbor (i+1, wrap)
    nc.vector.tensor_tensor(o[:, : NX - 1, :], o[:, : NX - 1, :], x[:, 1:, :], add)
    nc.vector.tensor_tensor(o[:, NX - 1, :], o[:, NX - 1, :], x[:, 0, :], add)
    # left neighbor (j-1, wrap)
    nc.vector.tensor_tensor(o[:, :, 1:], o[:, :, 1:], x[:, :, : NX - 1], add)
    nc.vector.tensor_tensor(o[:, :, 0], o[:, :, 0], x[:, :, NX - 1], add)
    # right neighbor (j+1, wrap)
    nc.vector.tensor_tensor(o[:, :, : NX - 1], o[:, :, : NX - 1], x[:, :, 1:], add)
    nc.vector.tensor_tensor(o[:, :, NX - 1], o[:, :, NX - 1], x[:, :, 0], add)


def _build_program():
    import concourse.mybir as mybir
    import concourse.tile as tile
    from concourse import bacc

    bf16 = mybir.dt.bfloat16
    f32 = mybir.dt.float32
    relu = mybir.ActivationFunctionType.Relu

    nc = bacc.Bacc("TRN2", target_bir_lowering=False, debug=False)

    x_t = nc.dram_tensor("x_t", [128, NODES_PER_CORE], bf16, kind="ExternalInput").ap()
    w1 = nc.dram_tensor("w1", [128, 2 * 2 * 128], bf16, kind="ExternalInput").ap()
    w2 = nc.dram_tensor("w2", [128, 4 * 2 * 128], bf16, kind="ExternalInput").ap()
    b1d = nc.dram_tensor("b1", [128, 2], f32, kind="ExternalInput").ap()
    b2d = nc.dram_tensor("b2", [128, 2], f32, kind="ExternalInput").ap()
    out_t = nc.dram_tensor(
        "out_t", [2, 128, NODES_PER_CORE], f32, kind="ExternalOutput"
    ).ap()

    with tile.TileContext(nc) as tc:
        with (
            tc.tile_pool(name="consts", bufs=1) as cpool,
            tc.tile_pool(name="xin", bufs=2) as xpool,
            tc.tile_pool(name="work", bufs=1) as wpool,
            tc.tile_pool(name="stage", bufs=4) as spool,
            tc.tile_pool(name="psum", bufs=8, space="PSUM") as ppool,
        ):
            w1_sb = cpool.tile([128, 2, 2, 128], bf16)
            nc.sync.dma_start(w1_sb[:], w1.rearrange("p (k m f) -> p k m f", k=2, m=2))
            w2_sb = cpool.tile([128, 4, 2, 128], bf16)
            nc.sync.dma_start(w2_sb[:], w2.rearrange("p (k m f) -> p k m f", k=4, m=2))
            b1_sb = [cpool.tile([128, 1], f32, name=f"b1_{m}") for m in range(2)]
            b2_sb = [cpool.tile([128, 1], f32, name=f"b2_{m}") for m in range(2)]
            for m in range(2):
                nc.sync.dma_start(b1_sb[m][:], b1d[:, m : m + 1])
                nc.sync.dma_start(b2_sb[m][:], b2d[:, m : m + 1])

            for t in range(TILES_PER_CORE):
                X = xpool.tile([128, NODES_PER_TILE], bf16, tag="X")
                nc.sync.dma_start(
                    X[:], x_t[:, t * NODES_PER_TILE : (t + 1) * NODES_PER_TILE]
                )
                XN = wpool.tile([128, NODES_PER_TILE], bf16, tag="XN")
                _stencil(nc, mybir, XN, X)

                H = [
                    wpool.tile([128, NODES_PER_TILE], bf16, tag=f"H{m}", name=f"H{m}")
                    for m in range(2)
                ]
                rhs1 = [X, XN]
                # ---- layer 1: K = 2 blocks (X, XN), M = 2 out blocks ----
                for g0 in range(0, N_CHUNKS, GROUP):
                    cs = range(g0, min(g0 + GROUP, N_CHUNKS))
                    ps = {
                        (c, m): ppool.tile([128, CHUNK], f32, tag="ps", name="ps1")
                        for c in cs
                        for m in range(2)
                    }
                    for k in range(2):
                        for m in range(2):
                            for c in cs:
                                nc.tensor.matmul(
                                    ps[(c, m)],
                                    w1_sb[:, k, m],
                                    rhs1[k][:, c * CHUNK : (c + 1) * CHUNK],
                                    start=(k == 0),
                                    stop=(k == 1),
                                )
                    for (c, m), p in ps.items():
                        nc.scalar.activation(
                            H[m][:, c * CHUNK : (c + 1) * CHUNK],
                            p,
                            relu,
                            bias=b1_sb[m][:, 0:1],
                        )

                HN = [
                    wpool.tile(
                        [128, NODES_PER_TILE], bf16, tag=f"HN{m}", name=f"HN{m}"
                    )
                    for m in range(2)
                ]
                _stencil(nc, mybir, HN[0], H[0])
                _stencil(nc, mybir, HN[1], H[1])

                rhs2 = [H[0], H[1], HN[0], HN[1]]
                # ---- layer 2: K = 4 blocks, M = 2 out blocks ----
                for g0 in range(0, N_CHUNKS, GROUP):
                    cs = range(g0, min(g0 + GROUP, N_CHUNKS))
                    ps = {
                        (c, m): ppool.tile([128, CHUNK], f32, tag="ps", name="ps2")
                        for c in cs
                        for m in range(2)
                    }
                    for k in range(4):
                        for m in range(2):
                            for c in cs:
                                nc.tensor.matmul(
                                    ps[(c, m)],
                                    w2_sb[:, k, m],
                                    rhs2[k][:, c * CHUNK : (c + 1) * CHUNK],
                                    start=(k == 0),
                                    stop=(k == 3),
                                )
                    for (c, m), p in ps.items():
                        o = spool.tile([128, CHUNK], f32, tag="ostage")
                        nc.scalar.activation(o[:], p, relu, bias=b2_sb[m][:, 0:1])
                        off = t * NODES_PER_TILE + c * CHUNK
                        nc.sync.dma_start(out_t[m, :, off : off + CHUNK], o[:])
    nc.compile()
    return nc


def _get_program():
    global _cached_nc
    if _cached_nc is None:
        _cached_nc = _build_program()
    return _cached_nc


def _make_in_maps(x, W_self1, W_neigh1, b1, W_self2, W_neigh2, b2):
    f32 = np.float32
    W1 = np.concatenate(
        [np.asarray(W_self1, f32), 0.25 * np.asarray(W_neigh1, f32)], axis=0
    )  # [256, 256]
    w1_host = np.ascontiguousarray(
        W1.reshape(2, 128, 2, 128).transpose(1, 0, 2, 3).reshape(128, 512)
    ).astype(_BF16)
    W2 = np.concatenate(
        [np.asarray(W_self2, f32), 0.25 * np.asarray(W_neigh2, f32)], axis=0
    )  # [512, 256]
    w2_host = np.ascontiguousarray(
        W2.reshape(4, 128, 2, 128).transpose(1, 0, 2, 3).reshape(128, 1024)
    ).astype(_BF16)
    b1_host = np.ascontiguousarray(np.asarray(b1, f32).reshape(2, 128).T)
    b2_host = np.ascontiguousarray(np.asarray(b2, f32).reshape(2, 128).T)

    x = np.asarray(x, f32)
    in_maps = []
    for core in range(N_CORES):
        b_, h_ = divmod(core, 2)
        xs = x[b_, h_ * TILES_PER_CORE : (h_ + 1) * TILES_PER_CORE].reshape(-1, IN_C)
        x_t = np.ascontiguousarray(xs.T).astype(_BF16)  # [128, 27648]
        in_maps.append(
            {
                "x_t": x_t,
                "w1": w1_host,
                "w2": w2_host,
                "b1": b1_host,
                "b2": b2_host,
            }
        )
    return in_maps


def _assemble_output(results):
    out = np.empty((BATCH, N_TILES, NX, NX, HID_C), np.float32)
    for core in range(N_CORES):
        b_, h_ = divmod(core, 2)
        o = results[core]["out_t"].reshape(HID_C, TILES_PER_CORE, NX, NX)
        out[b_, h_ * TILES_PER_CORE : (h_ + 1) * TILES_PER_CORE] = o.transpose(
            1, 2, 3, 0
        )
    return out


def _run(inputs, trace=False):
    """Run on the 8 NeuronCores; returns (output, BassKernelResults)."""
    from concourse.bass_utils import run_bass_kernel_spmd

    in_maps = _make_in_maps(
        inputs["x"],
        inputs["W_self1"],
        inputs["W_neigh1"],
        inputs["b1"],
        inputs["W_self2"],
        inputs["W_neigh2"],
        inputs["b2"],
    )
    nc = _get_program()
    res = run_bass_kernel_spmd(nc, in_maps, list(range(N_CORES)), trace=trace)
    return _assemble_output(res.results), res


def kernel(**inputs) -> np.ndarray:
    neighbors = np.asarray(inputs["neighbors"])
    if not np.array_equal(neighbors, _build_grid_neighbors()):
        # Graph is not the reference periodic grid: fall back to exact host math.
        return _numpy_fallback(
            np.asarray(inputs["x"]),
            neighbors,
            np.asarray(inputs["W_self1"]),
            np.asarray(inputs["W_neigh1"]),
            np.asarray(inputs["b1"]),
            np.asarray(inputs["W_self2"]),
            np.asarray(inputs["W_neigh2"]),
            np.asarray(inputs["b2"]),
        )
    out, _ = _run(inputs, trace=False)
    return out
